# revision 29
# baseline (speedup 1.0000x reference)
"""MoE model (embed -> gate -> 4 dense experts -> softmax combine) on 8 TRN2 cores.

Data-parallel: batch (65536 tokens) sharded 8192/core; embedding tables,
expert weights, and gating weights replicated on every core (SBUF-resident,
bf16). All on-chip activations are kept feature-major ("transposed") so that
every matmul consumes operands in their natural layout:

  e_T[f, t]   = one-hot(vocab) matmul against the embedding tables
  h_T[d, t]   = silu(W1[e].T-tiles @ e_T + b1)        (PSUM fp32, evac bf16)
  eo_T[o, t]  = W2[e].T-tiles @ h_T + b2              (PSUM fp32)
  logits[e,t] = Wg.T-tiles @ e_T + bg ; softmax via exp / sum (unnormalized
                weights combined first, one reciprocal row scale at the end)
  out_T[o, t] = (sum_e exp_e * eo_e) * recip          (DVE, fp32)

Output per core is [128, 8192] (feature-major); host transposes on unshard.
"""

import os
import numpy as np
import ml_dtypes

import concourse.bass as bass
import concourse.mybir as mybir
import concourse.tile as tile
from concourse.vector_clock import ScopedClock, VectorClock
from concourse.bass_utils import run_bass_kernel_spmd

BF16 = ml_dtypes.bfloat16

B = 65536
V = 512
D = 1024
IN = 2048
E = 4
OUT = 128
NCORES = 8
BL = B // NCORES          # tokens per core
ST = 512                  # tokens per supertile (max PSUM free dim, fp32)
NST = BL // ST            # supertiles per core
KC = IN // 128            # 16 feature chunks
DC = D // 128             # 8 hidden chunks
VC = V // 128             # 4 vocab chunks

LAST_EXEC_NS = None       # set when BASSMOE_TRACE=1


class _TC(tile.TileContext):
    """Unmodified TileContext; kept as a named subclass for clarity."""


def _legalize_waits(nc, max_waits=1):
    """This walrus build rejects instructions carrying more than ~1 sync-wait
    command ("Too many sync wait commands", CoreV2/V3GenImpl setupSyncWait).
    Hoist all but the last wait of every instruction onto single-wait NoOps
    placed immediately before it in the same engine's stream."""
    for f in nc.m.functions:
        for bb in f.blocks:
            insts = bb.instructions
            if not any(
                inst.sync_info is not None and len(inst.sync_info.on_wait) > max_waits
                for inst in insts
            ):
                continue
            new = []
            for inst in insts:
                si = inst.sync_info
                waits = list(si.on_wait) if si is not None else []
                if len(waits) > max_waits:
                    for w in waits[:-max_waits]:
                        nop = mybir.InstNoOp(
                            name=f"legw-{nc.next_id()}", ins=[], outs=[]
                        )
                        nop.engine = inst.engine
                        nop.sync_info = mybir.SyncInfo(on_wait=[w], on_update=[])
                        new.append(nop)
                    inst.sync_info = mybir.SyncInfo(
                        on_wait=waits[-max_waits:], on_update=list(si.on_update)
                    )
                new.append(inst)
            bb.instructions = new


def build_program(nst=NST, legalize=True):
    dt = mybir.dt
    f32, bf16, f16 = dt.float32, dt.bfloat16, dt.float16
    AF = mybir.ActivationFunctionType
    ALU = mybir.AluOpType

    nc = bass.Bass()

    x0d = nc.dram_tensor("x0", [nst, 1, ST], f16, kind="ExternalInput")
    x1d = nc.dram_tensor("x1", [nst, 1, ST], f16, kind="ExternalInput")
    embd = nc.dram_tensor("embs", [128, 2, VC, DC, 128], bf16, kind="ExternalInput")
    w1d = nc.dram_tensor("w1s", [128, E, KC, DC, 128], bf16, kind="ExternalInput")
    w2d = nc.dram_tensor("w2s", [128, E, DC, OUT], bf16, kind="ExternalInput")
    wgd = nc.dram_tensor("wgs", [128, KC, E], bf16, kind="ExternalInput")
    b1d = nc.dram_tensor("b1s", [128, E, DC], f32, kind="ExternalInput")
    b2d = nc.dram_tensor("b2s", [128, E], f32, kind="ExternalInput")
    bgd = nc.dram_tensor("bgs", [E, 1], f32, kind="ExternalInput")
    ivd = nc.dram_tensor("ivs", [128, VC], f32, kind="ExternalInput")
    seld = nc.dram_tensor("sels", [E, E, 128], bf16, kind="ExternalInput")
    outd = nc.dram_tensor("out", [128, nst * ST], f32, kind="ExternalOutput")

    with _TC(nc) as tc:
        with (
            tc.tile_pool(name="const", bufs=1) as cpool,
            tc.tile_pool(name="xt", bufs=2) as xpool,
            tc.tile_pool(name="mask", bufs=1) as mpool,
            tc.tile_pool(name="et", bufs=1) as etpool,
            tc.tile_pool(name="hs", bufs=1) as hpool,
            tc.tile_pool(name="sm", bufs=2) as smpool,
            tc.tile_pool(name="gsc", bufs=1) as gspool,
            tc.tile_pool(name="sgp", bufs=2) as sgpool,
            tc.tile_pool(name="accp", bufs=1) as apool,
            tc.tile_pool(name="outp", bufs=2) as opool,
            tc.tile_pool(name="pmm", bufs=2, space="PSUM") as pmm,
            tc.tile_pool(name="peo", bufs=2, space="PSUM") as peo,
            tc.tile_pool(name="prb", bufs=2, space="PSUM") as prb,
            tc.tile_pool(name="pmisc", bufs=2, space="PSUM") as pmisc,
        ):
            # --- resident weights / constants ---
            emb_sb = cpool.tile([128, 2, VC, DC, 128], bf16)
            nc.sync.dma_start(emb_sb[:], embd[:])
            w1_sb = cpool.tile([128, E, KC, DC, 128], bf16)
            nc.sync.dma_start(w1_sb[:], w1d[:])
            w2_sb = cpool.tile([128, E, DC, OUT], bf16)
            nc.sync.dma_start(w2_sb[:], w2d[:])
            wg_sb = cpool.tile([128, KC, E], bf16)
            nc.sync.dma_start(wg_sb[:], wgd[:])
            b1_sb = cpool.tile([128, E, DC], f32)
            nc.sync.dma_start(b1_sb[:], b1d[:])
            b2_sb = cpool.tile([128, E], f32)
            nc.sync.dma_start(b2_sb[:], b2d[:])
            bg_sb = cpool.tile([E, 1], f32)
            nc.sync.dma_start(bg_sb[:], bgd[:])
            iv_sb = cpool.tile([128, VC], f32)
            nc.sync.dma_start(iv_sb[:], ivd[:])
            sel_sb = cpool.tile([E, E, 128], bf16)
            nc.sync.dma_start(sel_sb[:], seld[:])

            ones_f16 = cpool.tile([1, 128], f16)
            nc.vector.memset(ones_f16[:], 1.0)
            ones4_bf = cpool.tile([E, 1], bf16)
            nc.vector.memset(ones4_bf[:], 1.0)
            ones128_bf = cpool.tile([1, 128], bf16)
            nc.vector.memset(ones128_bf[:], 1.0)

            for i in range(nst):
                # --- x broadcast across partitions (K=1 matmul) ---
                x0_sb = xpool.tile([1, ST], f16, tag="x0")
                nc.sync.dma_start(x0_sb[:], x0d[i])
                x1_sb = xpool.tile([1, ST], f16, tag="x1")
                nc.sync.dma_start(x1_sb[:], x1d[i])
                xb = []
                for tbl, xs in enumerate((x0_sb, x1_sb)):
                    p = pmisc.tile([128, ST], f32, tag="misc")
                    nc.tensor.matmul(p[:], ones_f16[:], xs[:])
                    xb.append(p)

                # --- one-hot masks + embedding matmul -> e_T ---
                eT = etpool.tile([128, KC, ST], bf16, tag="eT")
                for tbl in range(2):
                    masks = []
                    for vc in range(VC):
                        m = mpool.tile([128, ST], bf16, tag=f"m{vc}")
                        nc.vector.tensor_scalar(
                            m[:], xb[tbl][:], iv_sb[:, vc : vc + 1], None, ALU.is_equal
                        )
                        masks.append(m)
                    for dc in range(DC):
                        ps = pmm.tile([128, ST], f32, tag="mm")
                        for vc in range(VC):
                            nc.tensor.matmul(
                                ps[:],
                                emb_sb[:, tbl, vc, dc, :],
                                masks[vc][:],
                                start=(vc == 0),
                                stop=(vc == VC - 1),
                            )
                        nc.scalar.copy(eT[:, tbl * DC + dc, :], ps[:])

                # --- gating: logits -> exp -> sum -> reciprocal bcast ---
                lp = pmisc.tile([E, ST], f32, tag="misc")
                for kc in range(KC):
                    nc.tensor.matmul(
                        lp[:],
                        wg_sb[:, kc, :],
                        eT[:, kc, :],
                        start=(kc == 0),
                        stop=(kc == KC - 1),
                    )
                expt = smpool.tile([E, ST], bf16, tag="expt")
                nc.scalar.activation(expt[:], lp[:], AF.Exp, bias=bg_sb[:])
                sp = pmisc.tile([1, ST], f32, tag="misc")
                nc.tensor.matmul(sp[:], ones4_bf[:], expt[:])
                rec = smpool.tile([1, ST], f32, tag="rec")
                nc.vector.reciprocal(rec[:], sp[:])
                recb = smpool.tile([1, ST], bf16, tag="recb")
                nc.vector.tensor_copy(recb[:], rec[:])
                rbp = prb.tile([128, ST], f32, tag="rb")
                nc.tensor.matmul(rbp[:], ones128_bf[:], recb[:])

                # --- experts ---
                acc = apool.tile([128, ST], f32, tag="acc")
                for e in range(E):
                    hs = hpool.tile([128, DC, ST], bf16, tag="hs")
                    for dc in range(DC):
                        hp = pmm.tile([128, ST], f32, tag="mm")
                        for kc in range(KC):
                            nc.tensor.matmul(
                                hp[:],
                                w1_sb[:, e, kc, dc, :],
                                eT[:, kc, :],
                                start=(kc == 0),
                                stop=(kc == KC - 1),
                            )
                        sg = sgpool.tile([128, ST], f32, tag="sg")
                        nc.scalar.activation(
                            sg[:], hp[:], AF.Sigmoid, bias=b1_sb[:, e, dc : dc + 1]
                        )
                        nc.vector.scalar_tensor_tensor(
                            hs[:, dc, :], hp[:], b1_sb[:, e, dc : dc + 1], sg[:],
                            ALU.add, ALU.mult,
                        )
                    eop = peo.tile([128, ST], f32, tag="eo")
                    for dc in range(DC):
                        nc.tensor.matmul(
                            eop[:],
                            w2_sb[:, e, dc, :],
                            hs[:, dc, :],
                            start=(dc == 0),
                            stop=(dc == DC - 1),
                        )
                    gp = pmisc.tile([128, ST], f32, tag="misc")
                    nc.tensor.matmul(gp[:], sel_sb[:, e, :], expt[:])
                    gs = gspool.tile([128, ST], f32, tag="gs")
                    nc.scalar.copy(gs[:], gp[:])
                    if e == 0:
                        nc.vector.scalar_tensor_tensor(
                            acc[:], eop[:], b2_sb[:, e : e + 1], gs[:], ALU.add, ALU.mult
                        )
                    else:
                        tmp = opool.tile([128, ST], f32, tag="outt")
                        nc.vector.scalar_tensor_tensor(
                            tmp[:], eop[:], b2_sb[:, e : e + 1], gs[:], ALU.add, ALU.mult
                        )
                        nc.vector.tensor_add(acc[:], acc[:], tmp[:])

                outt = opool.tile([128, ST], f32, tag="outt")
                nc.vector.tensor_tensor(outt[:], acc[:], rbp[:], ALU.mult)
                nc.sync.dma_start(outd[:, i * ST : (i + 1) * ST], outt[:])

    if legalize:
        _legalize_waits(nc)
    return nc


def marshal_inputs(x, emb0, emb1, W1, b1, W2, b2, Wg, bg, nst=NST, ncores=NCORES):
    """Host-side: cast/reshape full inputs into per-core in_maps."""
    n_tok = ncores * nst * ST
    x0h = np.ascontiguousarray(
        x[:n_tok, 0].astype(np.float16).reshape(ncores, nst, 1, ST)
    )
    x1h = np.ascontiguousarray(
        x[:n_tok, 1].astype(np.float16).reshape(ncores, nst, 1, ST)
    )
    embs = np.ascontiguousarray(
        np.stack([emb0, emb1])
        .reshape(2, VC, 128, DC, 128)
        .transpose(2, 0, 1, 3, 4)
        .astype(BF16)
    )
    w1s = np.ascontiguousarray(
        np.asarray(W1).reshape(E, KC, 128, DC, 128).transpose(2, 0, 1, 3, 4).astype(BF16)
    )
    w2s = np.ascontiguousarray(
        np.asarray(W2).reshape(E, DC, 128, OUT).transpose(2, 0, 1, 3).astype(BF16)
    )
    wgs = np.ascontiguousarray(
        np.asarray(Wg).reshape(KC, 128, E).transpose(1, 0, 2).astype(BF16)
    )
    b1s = np.ascontiguousarray(
        np.asarray(b1).reshape(E, DC, 128).transpose(2, 0, 1).astype(np.float32)
    )
    b2s = np.ascontiguousarray(np.asarray(b2).T.astype(np.float32))
    bgs = np.ascontiguousarray(np.asarray(bg).reshape(E, 1).astype(np.float32))
    ivs = np.ascontiguousarray(
        (np.arange(VC)[None, :] * 128 + np.arange(128)[:, None]).astype(np.float32)
    )
    sels = np.ascontiguousarray(
        np.broadcast_to(np.eye(E, dtype=np.float32)[:, :, None], (E, E, 128)).astype(
            BF16
        )
    )
    shared = {
        "embs": embs, "w1s": w1s, "w2s": w2s, "wgs": wgs,
        "b1s": b1s, "b2s": b2s, "bgs": bgs, "ivs": ivs, "sels": sels,
    }
    return [{"x0": x0h[c], "x1": x1h[c], **shared} for c in range(ncores)]


def kernel(x, emb0, emb1, W1, b1, W2, b2, Wg, bg):
    global LAST_EXEC_NS
    nc = build_program()
    in_maps = marshal_inputs(x, emb0, emb1, W1, b1, W2, b2, Wg, bg)
    trace = os.environ.get("BASSMOE_TRACE", "0") == "1"
    res = run_bass_kernel_spmd(nc, in_maps, list(range(NCORES)), trace=trace)
    LAST_EXEC_NS = res.exec_time_ns
    out = np.empty((B, OUT), dtype=np.float32)
    for c in range(NCORES):
        out[c * BL : (c + 1) * BL, :] = res.results[c]["out"].T
    return out
